# revision 16
# baseline (speedup 1.0000x reference)
"""Trainium2 Bass kernel for nn_EstimateAdj (GNN message passing).

reference semantics:
    h = relu(features @ W1 + b1)            # [N, H]
    r = h @ W2 + b2                         # [N, H]  ("representations")
    total_edge_index = concat(edge_index, pred_edge_index, axis=1)
    w[e] = relu(dot(r[src_e], r[dst_e]))    # only needed for the original
    predictor_weights = w[:E_ORIG]          #   E_ORIG edges
    returns (r, predictor_weights, total_edge_index, edge_index)

Device strategy (8 cores, SPMD):
  - The MLP is replicated: every core computes the full [N, H] table (it
    needs random access to every row for its edge shard anyway) and writes
    it to an HBM output tensor `r_full`.
  - Edges are sharded 1/8 per core.  Rows are bulk-gathered with the SWDGE
    `dma_gather` op, whose indices are int16: the node table is split into
    4 chunks of 25000 rows and each core's edges are host-sorted into the
    16 (src_chunk, dst_chunk) groups; local (within-chunk) indices then fit
    int16.  Gathered x0/x1 tiles are multiplied on DVE and seg-reduced over
    H; the host applies the inverse edge permutation to the result.
  - total_edge_index / edge_index outputs are pure host-side concat /
    passthrough of inputs.
"""

import numpy as np

import concourse.bacc as bacc
import concourse.bass as bass
import concourse.mybir as mybir
from concourse.bass_utils import run_bass_kernel_spmd
from concourse.masks import make_identity
from concourse.tile import TileContext

F32 = mybir.dt.float32
I16 = mybir.dt.int16

N_CORES = 8

FULL_CFG = dict(
    n=100000,       # nodes
    nfea=128,       # input features
    h=64,           # hidden/out dim
    e_pc=200000,    # edges per core (original edges only)
    chunk=25000,    # node-table chunk (< 2**15 so local idx fits int16)
    nb_max=1024,    # gather rows per dma_gather instruction (ring limit)
    node_chunk=512,
    mlp_dtype="f32r",  # f32 | f32r
)


# --------------------------------------------------------------------------
# host-side edge grouping
# --------------------------------------------------------------------------

def plan_edges(cfg, edge_index):
    """Sort each core's edge shard by (src_chunk, dst_chunk); build packed
    int16 index arrays and the info needed to unscramble the results."""
    n, e_pc, chunk = cfg["n"], cfg["e_pc"], cfg["chunk"]
    nchunk = (n + chunk - 1) // chunk
    ngroups = nchunk * nchunk

    src_all = np.asarray(edge_index[0], dtype=np.int64)
    dst_all = np.asarray(edge_index[1], dtype=np.int64)

    per_core = []
    counts = np.zeros((N_CORES, ngroups), dtype=np.int64)
    for c in range(N_CORES):
        lo = c * e_pc
        src = src_all[lo: lo + e_pc]
        dst = dst_all[lo: lo + e_pc]
        g = (src // chunk) * nchunk + dst // chunk
        order = np.argsort(g, kind="stable")
        counts[c] = np.bincount(g, minlength=ngroups)
        per_core.append((src, dst, g, order))

    caps = counts.max(axis=0)
    caps = ((caps + 127) // 128) * 128          # group slots, 128-aligned
    offs = np.zeros(ngroups + 1, dtype=np.int64)
    np.cumsum(caps, out=offs[1:])
    cap_total = int(offs[-1])

    src_packed = np.zeros((N_CORES, 128, cap_total // 16), dtype=np.int16)
    dst_packed = np.zeros_like(src_packed)
    unpack = []  # per core: (order, valid_pos)
    for c in range(N_CORES):
        src, dst, g, order = per_core[c]
        s_sorted = np.zeros(cap_total, dtype=np.int16)
        d_sorted = np.zeros(cap_total, dtype=np.int16)
        valid_pos = np.empty(e_pc, dtype=np.int64)
        pos = 0
        for gi in range(ngroups):
            cnt = int(counts[c, gi])
            if cnt:
                sel = order[pos: pos + cnt]
                o = int(offs[gi])
                s_sorted[o: o + cnt] = (src[sel] - (gi // nchunk) * chunk).astype(
                    np.int16
                )
                d_sorted[o: o + cnt] = (dst[sel] - (gi % nchunk) * chunk).astype(
                    np.int16
                )
                valid_pos[pos: pos + cnt] = o + np.arange(cnt)
                pos += cnt
        # wrap: index k lives at [k % 16, k // 16]; replicate to all 128 rows
        wrapped_s = s_sorted.reshape(-1, 16).T
        wrapped_d = d_sorted.reshape(-1, 16).T
        src_packed[c] = np.tile(wrapped_s, (8, 1))
        dst_packed[c] = np.tile(wrapped_d, (8, 1))
        unpack.append((order, valid_pos))

    groups = [
        (gi // nchunk, gi % nchunk, int(offs[gi]), int(caps[gi]))
        for gi in range(ngroups)
        if caps[gi] > 0
    ]
    return dict(
        groups=groups,
        cap_total=cap_total,
        src_packed=src_packed,
        dst_packed=dst_packed,
        unpack=unpack,
        nchunk=nchunk,
    )


# --------------------------------------------------------------------------
# kernel build
# --------------------------------------------------------------------------

def build_kernel(cfg, groups, cap_total):
    n, nfea, h = cfg["n"], cfg["nfea"], cfg["h"]
    chunk, nb_max = cfg["chunk"], cfg["nb_max"]
    node_chunk = cfg["node_chunk"]
    assert nfea <= 128 and h <= 128

    use_f32r = cfg["mlp_dtype"] == "f32r"
    mm_dt = mybir.dt.float32r if use_f32r else F32

    nc = bacc.Bacc(
        "TRN2",
        target_bir_lowering=False,
        debug=False,
        num_devices=N_CORES,
        num_swdge_queues=4,
    )

    features = nc.dram_tensor("features", [n, nfea], F32, kind="ExternalInput")
    w1_d = nc.dram_tensor("w1", [nfea, h], F32, kind="ExternalInput")
    b1_d = nc.dram_tensor("b1", [h, 1], F32, kind="ExternalInput")
    w2_d = nc.dram_tensor("w2", [h, h], F32, kind="ExternalInput")
    b2_d = nc.dram_tensor("b2", [h, 1], F32, kind="ExternalInput")
    src_d = nc.dram_tensor(
        "src_idx", [128, cap_total // 16], I16, kind="ExternalInput"
    )
    dst_d = nc.dram_tensor(
        "dst_idx", [128, cap_total // 16], I16, kind="ExternalInput"
    )

    r_full = nc.dram_tensor("r_full", [n, h], F32, kind="ExternalOutput")
    w_out = nc.dram_tensor("w_out", [128, cap_total // 128], F32,
                           kind="ExternalOutput")

    with TileContext(nc) as tc:
        with (
            tc.tile_pool(name="consts", bufs=1) as cpool,
            tc.tile_pool(name="mlp", bufs=3) as mpool,
            tc.tile_pool(name="mlp_ps", bufs=2, space="PSUM") as mpsum,
            tc.tile_pool(name="edge", bufs=3) as epool,
            tc.tile_pool(name="wacc", bufs=1) as wpool,
        ):
            # ---- constants ------------------------------------------------
            ident = cpool.tile([128, 128], F32, tag="ident")
            make_identity(nc, ident[:])
            w1_s = cpool.tile([nfea, h], mm_dt, tag="w1")
            w2_s = cpool.tile([h, h], mm_dt, tag="w2")
            if use_f32r:
                w1_raw = cpool.tile([nfea, h], F32, tag="w1raw")
                nc.sync.dma_start(w1_raw[:], w1_d[:, :])
                nc.vector.tensor_copy(w1_s[:], w1_raw[:])
                w2_raw = cpool.tile([h, h], F32, tag="w2raw")
                nc.sync.dma_start(w2_raw[:], w2_d[:, :])
                nc.vector.tensor_copy(w2_s[:], w2_raw[:])
            else:
                nc.sync.dma_start(w1_s[:], w1_d[:, :])
                nc.sync.dma_start(w2_s[:], w2_d[:, :])
            b1_s = cpool.tile([h, 1], F32, tag="b1")
            nc.sync.dma_start(b1_s[:], b1_d[:, :])
            b2_s = cpool.tile([h, 1], F32, tag="b2")
            nc.sync.dma_start(b2_s[:], b2_d[:, :])

            # ---- phase A: MLP over all nodes ------------------------------
            for base in range(0, n, node_chunk):
                nch = min(node_chunk, n - base)
                nsub = (nch + 127) // 128

                f_tile = mpool.tile([128, nsub * nfea], F32, tag="f")
                for s in range(nsub):
                    m = min(128, nch - s * 128)
                    nc.sync.dma_start(
                        f_tile[:m, s * nfea:(s + 1) * nfea],
                        features[base + s * 128: base + s * 128 + m, :],
                    )

                fT_ps = mpsum.tile([128, node_chunk], F32, tag="fT_ps")
                for s in range(nsub):
                    m = min(128, nch - s * 128)
                    nc.tensor.transpose(
                        fT_ps[:nfea, s * 128: s * 128 + m],
                        f_tile[:m, s * nfea:(s + 1) * nfea],
                        ident[:m, :m],
                    )
                fT_s = mpool.tile([128, node_chunk], mm_dt, tag="fT_s")
                nc.vector.tensor_copy(fT_s[:nfea, :nch], fT_ps[:nfea, :nch])

                hT_ps = mpsum.tile([h, node_chunk], F32, tag="hT_ps")
                nc.tensor.matmul(
                    hT_ps[:, :nch], w1_s[:], fT_s[:nfea, :nch],
                    start=True, stop=True,
                )
                hT_s = mpool.tile([h, node_chunk], mm_dt, tag="hT_s")
                nc.scalar.activation(
                    hT_s[:, :nch], hT_ps[:, :nch],
                    mybir.ActivationFunctionType.Relu,
                    bias=b1_s[:, :1], scale=1.0,
                )

                rT_ps = mpsum.tile([h, node_chunk], F32, tag="rT_ps")
                nc.tensor.matmul(
                    rT_ps[:, :nch], w2_s[:], hT_s[:, :nch],
                    start=True, stop=True,
                )
                rT_s = mpool.tile([h, node_chunk], F32, tag="rT_s")
                nc.scalar.activation(
                    rT_s[:, :nch], rT_ps[:, :nch],
                    mybir.ActivationFunctionType.Identity,
                    bias=b2_s[:, :1], scale=1.0,
                )

                r_ps = mpsum.tile([128, (node_chunk // 128) * h], F32, tag="r_ps")
                for s in range(nsub):
                    m = min(128, nch - s * 128)
                    nc.tensor.transpose(
                        r_ps[:m, s * h:(s + 1) * h],
                        rT_s[:, s * 128: s * 128 + m],
                        ident[:h, :h],
                    )
                r_s = mpool.tile([128, (node_chunk // 128) * h], F32, tag="r_s")
                nc.vector.tensor_copy(r_s[:, :nsub * h], r_ps[:, :nsub * h])

                for s in range(nsub):
                    m = min(128, nch - s * 128)
                    nc.sync.dma_start(
                        r_full[base + s * 128: base + s * 128 + m, :],
                        r_s[:m, s * h:(s + 1) * h],
                    )

            # ---- phase B: per-edge gather / dot / relu --------------------
            w_acc = wpool.tile([128, cap_total // 128], F32, tag="w_acc")
            qn = 0
            for (ci, cj, off, cap) in groups:
                for q0 in range(0, cap, nb_max):
                    nb = min(nb_max, cap - q0)
                    pos = off + q0          # multiple of 128
                    ncol = nb // 128

                    sidx = epool.tile([128, nb_max // 16], I16, tag="sidx")
                    nc.sync.dma_start(
                        sidx[:, :nb // 16],
                        src_d[:, pos // 16: (pos + nb) // 16],
                    )
                    didx = epool.tile([128, nb_max // 16], I16, tag="didx")
                    nc.sync.dma_start(
                        didx[:, :nb // 16],
                        dst_d[:, pos // 16: (pos + nb) // 16],
                    )

                    x0 = epool.tile([128, (nb_max // 128) * h], F32, tag="x0")
                    nc.gpsimd.dma_gather(
                        out_ap=x0[:].rearrange(
                            "p (c h) -> p c h", h=h
                        )[:, :ncol, :],
                        in_ap=r_full[ci * chunk:(ci + 1) * chunk, :],
                        idxs_ap=sidx[:, :nb // 16],
                        num_idxs=nb,
                        num_idxs_reg=nb,
                        elem_size=h,
                        queue_num=qn % 4,
                    )
                    x1 = epool.tile([128, (nb_max // 128) * h], F32, tag="x1")
                    nc.gpsimd.dma_gather(
                        out_ap=x1[:].rearrange(
                            "p (c h) -> p c h", h=h
                        )[:, :ncol, :],
                        in_ap=r_full[cj * chunk:(cj + 1) * chunk, :],
                        idxs_ap=didx[:, :nb // 16],
                        num_idxs=nb,
                        num_idxs_reg=nb,
                        elem_size=h,
                        queue_num=(qn + 1) % 4,
                    )
                    qn += 2

                    nc.vector.tensor_mul(
                        x0[:, :ncol * h], x0[:, :ncol * h], x1[:, :ncol * h]
                    )
                    nc.vector.tensor_reduce(
                        w_acc[:, pos // 128: pos // 128 + ncol],
                        x0[:].rearrange("p (c h) -> p c h", h=h)[:, :ncol, :],
                        axis=mybir.AxisListType.X,
                        op=mybir.AluOpType.add,
                    )

            nc.scalar.activation(
                w_acc[:], w_acc[:], mybir.ActivationFunctionType.Relu,
            )
            nc.sync.dma_start(w_out[:, :], w_acc[:])

    nc.compile()
    return nc


# --------------------------------------------------------------------------
# host wrapper
# --------------------------------------------------------------------------

_NC_CACHE = {}


def _get_nc(cfg, groups, cap_total):
    key = (tuple(sorted(cfg.items())), tuple(groups), cap_total)
    if key not in _NC_CACHE:
        _NC_CACHE[key] = build_kernel(cfg, groups, cap_total)
    return _NC_CACHE[key]


def run_cfg(cfg, features, edge_index, W1, b1, W2, b2, **run_kwargs):
    plan = plan_edges(cfg, edge_index)
    nc = _get_nc(cfg, plan["groups"], plan["cap_total"])

    features = np.ascontiguousarray(features, dtype=np.float32)
    w1 = np.ascontiguousarray(W1, dtype=np.float32)
    w2 = np.ascontiguousarray(W2, dtype=np.float32)
    b1c = np.ascontiguousarray(np.asarray(b1, dtype=np.float32).reshape(-1, 1))
    b2c = np.ascontiguousarray(np.asarray(b2, dtype=np.float32).reshape(-1, 1))

    in_maps = [
        {
            "features": features,
            "w1": w1,
            "b1": b1c,
            "w2": w2,
            "b2": b2c,
            "src_idx": plan["src_packed"][c],
            "dst_idx": plan["dst_packed"][c],
        }
        for c in range(N_CORES)
    ]
    # The first NEFF load after an unclean prior process occasionally wedges
    # the accelerator (NRT_EXEC_UNIT_UNRECOVERABLE); the failed attempt
    # resets it, so retry a couple of times.
    last_err = None
    for _attempt in range(3):
        try:
            res = run_bass_kernel_spmd(
                nc, in_maps, core_ids=list(range(N_CORES)), **run_kwargs
            )
            break
        except Exception as e:  # noqa: BLE001
            last_err = e
            import time as _time

            _time.sleep(5.0)
    else:
        raise last_err

    e_pc = cfg["e_pc"]
    representations = np.asarray(res.results[0]["r_full"], dtype=np.float32)
    weights = np.empty(e_pc * N_CORES, dtype=np.float32)
    for c in range(N_CORES):
        w_sorted = res.results[c]["w_out"].T.reshape(-1)
        order, valid_pos = plan["unpack"][c]
        w_core = np.empty(e_pc, dtype=np.float32)
        w_core[order] = w_sorted[valid_pos]
        weights[c * e_pc:(c + 1) * e_pc] = w_core
    return representations, weights, res


def kernel(features, edge_index, pred_edge_index, W1, b1, W2, b2):
    cfg = FULL_CFG
    representations, weights, _ = run_cfg(
        cfg, features, edge_index, W1, b1, W2, b2
    )
    edge_index = np.asarray(edge_index)
    pred_edge_index = np.asarray(pred_edge_index)
    total_edge_index = np.concatenate([edge_index, pred_edge_index], axis=1)
    return (representations, weights, total_edge_index, edge_index)


# revision 20
# speedup vs baseline: 1.8588x; 1.8588x over previous
"""Trainium2 Bass kernel for nn_EstimateAdj (GNN message passing).

reference semantics:
    h = relu(features @ W1 + b1)            # [N, H]
    r = h @ W2 + b2                         # [N, H]  ("representations")
    total_edge_index = concat(edge_index, pred_edge_index, axis=1)
    w[e] = relu(dot(r[src_e], r[dst_e]))    # only needed for the original
    predictor_weights = w[:E_ORIG]          #   E_ORIG edges
    returns (r, predictor_weights, total_edge_index, edge_index)

Device strategy (8 cores, SPMD):
  - The MLP is replicated: every core computes the full [N, H] table (it
    needs random access to every row for its edge shard anyway) and writes
    it to an HBM output tensor `r_full`.
  - Edges are sharded 1/8 per core.  Rows are bulk-gathered with the SWDGE
    `dma_gather` op, whose indices are int16: the node table is split into
    4 chunks of 25000 rows and each core's edges are host-sorted into the
    16 (src_chunk, dst_chunk) groups; local (within-chunk) indices then fit
    int16.  Gathered x0/x1 tiles are multiplied on DVE and seg-reduced over
    H; the host applies the inverse edge permutation to the result.
  - total_edge_index / edge_index outputs are pure host-side concat /
    passthrough of inputs.
"""

import numpy as np

import concourse.bacc as bacc
import concourse.bass as bass
import concourse.mybir as mybir
from concourse.bass_utils import run_bass_kernel_spmd
from concourse.masks import make_identity
from concourse.tile import TileContext

F32 = mybir.dt.float32
I16 = mybir.dt.int16

N_CORES = 8

FULL_CFG = dict(
    n=100000,       # nodes
    nfea=128,       # input features
    h=64,           # hidden/out dim
    e_pc=200000,    # edges per core (original edges only)
    chunk=25000,    # node-table chunk (< 2**15 so local idx fits int16)
    nb_max=1024,    # gather rows per dma_gather instruction (ring limit)
    node_chunk=512,
    mlp_dtype="f32",   # f32 | f32r
)


# --------------------------------------------------------------------------
# host-side edge grouping
# --------------------------------------------------------------------------

def plan_edges(cfg, edge_index):
    """Sort each core's edge shard by (src_chunk, dst_chunk); build packed
    int16 index arrays and the info needed to unscramble the results."""
    n, e_pc, chunk = cfg["n"], cfg["e_pc"], cfg["chunk"]
    nchunk = (n + chunk - 1) // chunk
    ngroups = nchunk * nchunk

    src_all = np.asarray(edge_index[0], dtype=np.int64)
    dst_all = np.asarray(edge_index[1], dtype=np.int64)

    per_core = []
    counts = np.zeros((N_CORES, ngroups), dtype=np.int64)
    for c in range(N_CORES):
        lo = c * e_pc
        src = src_all[lo: lo + e_pc]
        dst = dst_all[lo: lo + e_pc]
        g = (src // chunk) * nchunk + dst // chunk
        order = np.argsort(g, kind="stable")
        counts[c] = np.bincount(g, minlength=ngroups)
        per_core.append((src, dst, g, order))

    caps = counts.max(axis=0)
    caps = ((caps + 127) // 128) * 128          # group slots, 128-aligned
    offs = np.zeros(ngroups + 1, dtype=np.int64)
    np.cumsum(caps, out=offs[1:])
    cap_total = int(offs[-1])

    src_packed = np.zeros((N_CORES, 128, cap_total // 16), dtype=np.int16)
    dst_packed = np.zeros_like(src_packed)
    unpack = []  # per core: (order, valid_pos)
    for c in range(N_CORES):
        src, dst, g, order = per_core[c]
        s_sorted = np.zeros(cap_total, dtype=np.int16)
        d_sorted = np.zeros(cap_total, dtype=np.int16)
        valid_pos = np.empty(e_pc, dtype=np.int64)
        pos = 0
        for gi in range(ngroups):
            cnt = int(counts[c, gi])
            if cnt:
                sel = order[pos: pos + cnt]
                o = int(offs[gi])
                s_sorted[o: o + cnt] = (src[sel] - (gi // nchunk) * chunk).astype(
                    np.int16
                )
                d_sorted[o: o + cnt] = (dst[sel] - (gi % nchunk) * chunk).astype(
                    np.int16
                )
                valid_pos[pos: pos + cnt] = o + np.arange(cnt)
                pos += cnt
        # wrap: index k lives at [k % 16, k // 16]; replicate to all 128 rows
        wrapped_s = s_sorted.reshape(-1, 16).T
        wrapped_d = d_sorted.reshape(-1, 16).T
        src_packed[c] = np.tile(wrapped_s, (8, 1))
        dst_packed[c] = np.tile(wrapped_d, (8, 1))
        unpack.append((order, valid_pos))

    groups = [
        (gi // nchunk, gi % nchunk, int(offs[gi]), int(caps[gi]))
        for gi in range(ngroups)
        if caps[gi] > 0
    ]
    return dict(
        groups=groups,
        cap_total=cap_total,
        src_packed=src_packed,
        dst_packed=dst_packed,
        unpack=unpack,
        nchunk=nchunk,
    )


# --------------------------------------------------------------------------
# kernel build
# --------------------------------------------------------------------------

def build_kernel(cfg, groups, cap_total):
    n, nfea, h = cfg["n"], cfg["nfea"], cfg["h"]
    chunk, nb_max = cfg["chunk"], cfg["nb_max"]
    node_chunk = cfg["node_chunk"]
    assert nfea <= 128 and h <= 128

    use_f32r = cfg["mlp_dtype"] == "f32r"
    mm_dt = mybir.dt.float32r if use_f32r else F32

    nc = bacc.Bacc(
        "TRN2",
        target_bir_lowering=False,
        debug=False,
        num_devices=N_CORES,
        num_swdge_queues=4,
    )

    features = nc.dram_tensor("features", [n, nfea], F32, kind="ExternalInput")
    w1_d = nc.dram_tensor("w1", [nfea, h], F32, kind="ExternalInput")
    b1_d = nc.dram_tensor("b1", [h, 1], F32, kind="ExternalInput")
    w2_d = nc.dram_tensor("w2", [h, h], F32, kind="ExternalInput")
    b2_d = nc.dram_tensor("b2", [h, 1], F32, kind="ExternalInput")
    src_d = nc.dram_tensor(
        "src_idx", [128, cap_total // 16], I16, kind="ExternalInput"
    )
    dst_d = nc.dram_tensor(
        "dst_idx", [128, cap_total // 16], I16, kind="ExternalInput"
    )

    r_full = nc.dram_tensor("r_full", [n, h], F32, kind="ExternalOutput")
    w_out = nc.dram_tensor("w_out", [128, cap_total // 128], F32,
                           kind="ExternalOutput")

    nchunk_tbl = (n + chunk - 1) // chunk

    with TileContext(nc) as tc:
        with (
            tc.tile_pool(name="consts", bufs=1) as cpool,
            tc.tile_pool(name="mlp", bufs=3) as mpool,
            tc.tile_pool(name="mlp_ps", bufs=2, space="PSUM") as mpsum,
            tc.tile_pool(name="edge", bufs=3) as epool,
            tc.tile_pool(name="wacc", bufs=1) as wpool,
        ):
            # ---- constants ------------------------------------------------
            ident = cpool.tile([128, 128], F32, tag="ident")
            make_identity(nc, ident[:])
            w1_s = cpool.tile([nfea, h], mm_dt, tag="w1")
            w2_s = cpool.tile([h, h], mm_dt, tag="w2")
            if use_f32r:
                w1_raw = cpool.tile([nfea, h], F32, tag="w1raw")
                nc.sync.dma_start(w1_raw[:], w1_d[:, :])
                nc.vector.tensor_copy(w1_s[:], w1_raw[:])
                w2_raw = cpool.tile([h, h], F32, tag="w2raw")
                nc.sync.dma_start(w2_raw[:], w2_d[:, :])
                nc.vector.tensor_copy(w2_s[:], w2_raw[:])
            else:
                nc.sync.dma_start(w1_s[:], w1_d[:, :])
                nc.sync.dma_start(w2_s[:], w2_d[:, :])
            b1_s = cpool.tile([h, 1], F32, tag="b1")
            nc.sync.dma_start(b1_s[:], b1_d[:, :])
            b2_s = cpool.tile([h, 1], F32, tag="b2")
            nc.sync.dma_start(b2_s[:], b2_d[:, :])

            # edge indices, resident for the whole kernel (loaded upfront so
            # gather issue is never stuck behind MLP DMAs in the SP stream)
            src_s = wpool.tile([128, cap_total // 16], I16, tag="src")
            nc.sync.dma_start(src_s[:], src_d[:, :])
            dst_s = wpool.tile([128, cap_total // 16], I16, tag="dst")
            nc.sync.dma_start(dst_s[:], dst_d[:, :])
            w_acc = wpool.tile([128, cap_total // 128], F32, tag="w_acc")

            qn = [0]

            def emit_mlp_chunk(base, nch):
                nsub = (nch + 127) // 128

                f_tile = mpool.tile([128, nsub * nfea], F32, tag="f")
                for s in range(nsub):
                    m = min(128, nch - s * 128)
                    nc.sync.dma_start(
                        f_tile[:m, s * nfea:(s + 1) * nfea],
                        features[base + s * 128: base + s * 128 + m, :],
                    )

                fT_ps = mpsum.tile([128, node_chunk], F32, tag="fT_ps")
                for s in range(nsub):
                    m = min(128, nch - s * 128)
                    nc.tensor.transpose(
                        fT_ps[:nfea, s * 128: s * 128 + m],
                        f_tile[:m, s * nfea:(s + 1) * nfea],
                        ident[:m, :m],
                    )
                fT_s = mpool.tile([128, node_chunk], mm_dt, tag="fT_s")
                nc.vector.tensor_copy(fT_s[:nfea, :nch], fT_ps[:nfea, :nch])

                hT_ps = mpsum.tile([h, node_chunk], F32, tag="hT_ps")
                nc.tensor.matmul(
                    hT_ps[:, :nch], w1_s[:], fT_s[:nfea, :nch],
                    start=True, stop=True,
                )
                hT_s = mpool.tile([h, node_chunk], mm_dt, tag="hT_s")
                nc.scalar.activation(
                    hT_s[:, :nch], hT_ps[:, :nch],
                    mybir.ActivationFunctionType.Relu,
                    bias=b1_s[:, :1], scale=1.0,
                )

                rT_ps = mpsum.tile([h, node_chunk], F32, tag="rT_ps")
                nc.tensor.matmul(
                    rT_ps[:, :nch], w2_s[:], hT_s[:, :nch],
                    start=True, stop=True,
                )
                rT_s = mpool.tile([h, node_chunk], F32, tag="rT_s")
                nc.scalar.activation(
                    rT_s[:, :nch], rT_ps[:, :nch],
                    mybir.ActivationFunctionType.Identity,
                    bias=b2_s[:, :1], scale=1.0,
                )

                r_ps = mpsum.tile([128, (node_chunk // 128) * h], F32, tag="r_ps")
                for s in range(nsub):
                    m = min(128, nch - s * 128)
                    nc.tensor.transpose(
                        r_ps[:m, s * h:(s + 1) * h],
                        rT_s[:, s * 128: s * 128 + m],
                        ident[:h, :h],
                    )
                r_s = mpool.tile([128, (node_chunk // 128) * h], F32, tag="r_s")
                nc.vector.tensor_copy(r_s[:, :nsub * h], r_ps[:, :nsub * h])

                for s in range(nsub):
                    m = min(128, nch - s * 128)
                    nc.sync.dma_start(
                        r_full[base + s * 128: base + s * 128 + m, :],
                        r_s[:m, s * h:(s + 1) * h],
                    )

            def emit_group(ci, cj, off, cap):
                for q0 in range(0, cap, nb_max):
                    nb = min(nb_max, cap - q0)
                    pos = off + q0          # multiple of 128
                    ncol = nb // 128

                    x0 = epool.tile([128, (nb_max // 128) * h], F32, tag="x0")
                    nc.gpsimd.dma_gather(
                        out_ap=x0[:].rearrange(
                            "p (c h) -> p c h", h=h
                        )[:, :ncol, :],
                        in_ap=r_full[ci * chunk:(ci + 1) * chunk, :],
                        idxs_ap=src_s[:, pos // 16: (pos + nb) // 16],
                        num_idxs=nb,
                        num_idxs_reg=nb,
                        elem_size=h,
                        queue_num=qn[0] % 4,
                    )
                    x1 = epool.tile([128, (nb_max // 128) * h], F32, tag="x1")
                    nc.gpsimd.dma_gather(
                        out_ap=x1[:].rearrange(
                            "p (c h) -> p c h", h=h
                        )[:, :ncol, :],
                        in_ap=r_full[cj * chunk:(cj + 1) * chunk, :],
                        idxs_ap=dst_s[:, pos // 16: (pos + nb) // 16],
                        num_idxs=nb,
                        num_idxs_reg=nb,
                        elem_size=h,
                        queue_num=(qn[0] + 1) % 4,
                    )
                    qn[0] += 2

                    nc.vector.tensor_mul(
                        x0[:, :ncol * h], x0[:, :ncol * h], x1[:, :ncol * h]
                    )
                    nc.vector.tensor_reduce(
                        w_acc[:, pos // 128: pos // 128 + ncol],
                        x0[:].rearrange("p (c h) -> p c h", h=h)[:, :ncol, :],
                        axis=mybir.AxisListType.X,
                        op=mybir.AluOpType.add,
                    )

            # interleave: MLP chunk-waves, each followed by the gather groups
            # they unlock (group (ci,cj) only needs table chunks ci and cj)
            for wave in range(nchunk_tbl):
                lo = wave * chunk
                hi = min((wave + 1) * chunk, n)
                for base in range(lo, hi, node_chunk):
                    emit_mlp_chunk(base, min(node_chunk, hi - base))
                for (ci, cj, off, cap) in groups:
                    if max(ci, cj) == wave:
                        emit_group(ci, cj, off, cap)

            nc.scalar.activation(
                w_acc[:], w_acc[:], mybir.ActivationFunctionType.Relu,
            )
            nc.sync.dma_start(w_out[:, :], w_acc[:])

    nc.compile()
    return nc


# --------------------------------------------------------------------------
# host wrapper
# --------------------------------------------------------------------------

_NC_CACHE = {}


def _get_nc(cfg, groups, cap_total):
    key = (tuple(sorted(cfg.items())), tuple(groups), cap_total)
    if key not in _NC_CACHE:
        _NC_CACHE[key] = build_kernel(cfg, groups, cap_total)
    return _NC_CACHE[key]


def run_cfg(cfg, features, edge_index, W1, b1, W2, b2, **run_kwargs):
    plan = plan_edges(cfg, edge_index)
    nc = _get_nc(cfg, plan["groups"], plan["cap_total"])

    features = np.ascontiguousarray(features, dtype=np.float32)
    w1 = np.ascontiguousarray(W1, dtype=np.float32)
    w2 = np.ascontiguousarray(W2, dtype=np.float32)
    b1c = np.ascontiguousarray(np.asarray(b1, dtype=np.float32).reshape(-1, 1))
    b2c = np.ascontiguousarray(np.asarray(b2, dtype=np.float32).reshape(-1, 1))

    in_maps = [
        {
            "features": features,
            "w1": w1,
            "b1": b1c,
            "w2": w2,
            "b2": b2c,
            "src_idx": plan["src_packed"][c],
            "dst_idx": plan["dst_packed"][c],
        }
        for c in range(N_CORES)
    ]
    # The first NEFF load after an unclean prior process occasionally wedges
    # the accelerator (NRT_EXEC_UNIT_UNRECOVERABLE); the failed attempt
    # resets it, so retry a couple of times.
    last_err = None
    for _attempt in range(3):
        try:
            res = run_bass_kernel_spmd(
                nc, in_maps, core_ids=list(range(N_CORES)), **run_kwargs
            )
            break
        except Exception as e:  # noqa: BLE001
            last_err = e
            import time as _time

            _time.sleep(5.0)
    else:
        raise last_err

    e_pc = cfg["e_pc"]
    representations = np.asarray(res.results[0]["r_full"], dtype=np.float32)
    weights = np.empty(e_pc * N_CORES, dtype=np.float32)
    for c in range(N_CORES):
        w_sorted = res.results[c]["w_out"].T.reshape(-1)
        order, valid_pos = plan["unpack"][c]
        w_core = np.empty(e_pc, dtype=np.float32)
        w_core[order] = w_sorted[valid_pos]
        weights[c * e_pc:(c + 1) * e_pc] = w_core
    return representations, weights, res


def kernel(features, edge_index, pred_edge_index, W1, b1, W2, b2):
    cfg = FULL_CFG
    representations, weights, _ = run_cfg(
        cfg, features, edge_index, W1, b1, W2, b2
    )
    edge_index = np.asarray(edge_index)
    pred_edge_index = np.asarray(pred_edge_index)
    total_edge_index = np.concatenate([edge_index, pred_edge_index], axis=1)
    return (representations, weights, total_edge_index, edge_index)
